# revision 9
# baseline (speedup 1.0000x reference)
"""Multi-head attention (B=4, S=2048, D=1024, H=16, hd=64) on 8 NeuronCores.

Tensor-parallel over heads: core c computes heads 2c, 2c+1. See kernel.py
docstring for the math. This version software-pipelines emission: batch
b+1's QKV projection and batch b's output projection are interleaved into
batch b's attention groups so the in-order PE queue has no idle phases.
"""

import sys

sys.path.insert(0, "/opt/trn_rl_repo")

import numpy as np
import concourse.bass as bass
import concourse.bacc as bacc
import concourse.mybir as mybir
import concourse.tile as tile
from concourse.bass_utils import run_bass_kernel_spmd

F32 = mybir.dt.float32
F32R = mybir.dt.float32r
AF = mybir.ActivationFunctionType

B, S, D = 4, 2048, 1024
SEQ = B * S
NCORES = 8
DPC = 128            # dims per core = 2 heads * 64
KT = D // 128        # 8 k-tiles for the QKV contraction
F = 512              # free-dim chunk
NSC = S // F         # seq chunks per batch = 4
NKB = S // 128       # key blocks per batch = 16
NQB = S // 128       # q blocks per batch = 16

_CACHE = {}


def _build():
    nc = bacc.Bacc("TRN2", target_bir_lowering=False, debug=False,
                   enable_asserts=False)

    xT_d = nc.dram_tensor("xT", [D, SEQ], F32R, kind="ExternalInput")
    wq_d = nc.dram_tensor("wqT", [D, DPC], F32R, kind="ExternalInput")
    wk_d = nc.dram_tensor("wkT", [D, DPC], F32R, kind="ExternalInput")
    wv_d = nc.dram_tensor("wvT", [D, DPC], F32R, kind="ExternalInput")
    wo_d = nc.dram_tensor("woT", [DPC, D], F32R, kind="ExternalInput")
    bq_d = nc.dram_tensor("bq", [DPC, 1], F32, kind="ExternalInput")
    bk_d = nc.dram_tensor("bk", [DPC, 1], F32, kind="ExternalInput")
    bv_d = nc.dram_tensor("bv", [DPC, 1], F32, kind="ExternalInput")
    ident_d = nc.dram_tensor("ident", [128, 128], F32R, kind="ExternalInput")
    ind0_d = nc.dram_tensor("indic0", [1, 128], F32R, kind="ExternalInput")
    ind1_d = nc.dram_tensor("indic1", [1, 128], F32R, kind="ExternalInput")
    ones_d = nc.dram_tensor("ones", [128, NKB], F32R, kind="ExternalInput")
    out_d = nc.dram_tensor("out", [SEQ, D], F32, kind="ExternalOutput")

    with tile.TileContext(nc) as tc:
        with (
            tc.tile_pool(name="wp", bufs=1) as wp,
            tc.tile_pool(name="xp", bufs=3) as xp,
            tc.tile_pool(name="qk", bufs=2) as qk,
            tc.tile_pool(name="vp", bufs=2) as vp,
            tc.tile_pool(name="vt", bufs=2) as vtp,
            tc.tile_pool(name="ap", bufs=8) as apool,
            tc.tile_pool(name="cx", bufs=2) as cxp,
            tc.tile_pool(name="zp", bufs=2) as zp,
            tc.tile_pool(name="op", bufs=6) as op,
            # PSUM bank budget (8 total): sc2 2x2 + qkv 1 + ct 2 + o 1
            tc.tile_pool(name="ps_sc", bufs=2, space=bass.MemorySpace.PSUM) as psb,
            tc.tile_pool(name="ps_q", bufs=1, space=bass.MemorySpace.PSUM) as psq,
            tc.tile_pool(name="ps_ct", bufs=2, space=bass.MemorySpace.PSUM) as psc,
            tc.tile_pool(name="ps_o", bufs=1, space=bass.MemorySpace.PSUM) as pso,
        ):
            # resident weights / constants
            wq_sb = wp.tile([128, KT, DPC], F32R, tag="wq")
            wk_sb = wp.tile([128, KT, DPC], F32R, tag="wk")
            wv_sb = wp.tile([128, KT, DPC], F32R, tag="wv")
            wo_sb = wp.tile([128, D], F32R, tag="wo")
            ident = wp.tile([128, 128], F32R, tag="id")
            ind0 = wp.tile([1, 128], F32R, tag="i0")
            ind1 = wp.tile([1, 128], F32R, tag="i1")
            ones = wp.tile([128, NKB], F32R, tag="on")
            bq_sb = wp.tile([DPC, 1], F32, tag="bq")
            bk_sb = wp.tile([DPC, 1], F32, tag="bk")
            bv_sb = wp.tile([DPC, 1], F32, tag="bv")
            nc.sync.dma_start(wq_sb[:],
                              wq_d[:].rearrange("(kt p) m -> p kt m", p=128))

            xT_r = xT_d[:].rearrange("(kt p) f -> p kt f", p=128)

            def load_rest_of_weights():
                nc.sync.dma_start(wk_sb[:],
                                  wk_d[:].rearrange("(kt p) m -> p kt m", p=128))
                nc.sync.dma_start(wv_sb[:],
                                  wv_d[:].rearrange("(kt p) m -> p kt m", p=128))
                nc.sync.dma_start(bq_sb[:], bq_d[:])
                nc.sync.dma_start(bk_sb[:], bk_d[:])
                nc.sync.dma_start(bv_sb[:], bv_d[:])
                nc.sync.dma_start(ident[:], ident_d[:])
                nc.sync.dma_start(ind0[:], ind0_d[:])
                nc.sync.dma_start(ind1[:], ind1_d[:])
                nc.sync.dma_start(ones[:], ones_d[:])
                nc.sync.dma_start(wo_sb[:], wo_d[:])

            st = [dict() for _ in range(B)]   # per-batch tiles: qt/kt/vaug/ctx

            # ---- phase-A unit builders (QKV projection for batch b) --------
            def a_units(b, defer_qt=False):
                units = []
                deferred = []
                dma_units = []
                fill_units = []

                def u_start():
                    st[b]["qt"] = qk.tile([128, S], F32R, tag="qt",
                                          name=f"qt{b}")
                    st[b]["kt"] = qk.tile([128, S], F32R, tag="kt",
                                          name=f"kt{b}")
                    # per key block: [h0 d(64) | ones | h1 d(64) | ones]
                    va = vp.tile([128, NKB, 130], F32R, tag="va",
                                 name=f"va{b}")
                    st[b]["va"] = va
                    nc.vector.tensor_copy(va[:, :, 64:65], ones[:].unsqueeze(2))
                    nc.vector.tensor_copy(va[:, :, 129:130], ones[:].unsqueeze(2))
                units.append(u_start)

                for sc in range(NSC):
                    def u_dma(sc=sc):
                        xt = xp.tile([128, KT, F], F32R, tag="xt",
                                     name=f"xt{b}_{sc}")
                        st[b][f"xt{sc}"] = xt
                        lo = b * S + sc * F
                        for k in range(KT):
                            nc.sync.dma_start(xt[:, k, :],
                                              xT_r[:, k, lo:lo + F])
                    dma_units.append(u_dma)

                    def u_fill_a(sc, which, w_sb):
                        xt = st[b][f"xt{sc}"]
                        ps = psq.tile([128, F], F32, tag="qkv",
                                      name=f"ps{b}_{sc}_{which}")
                        st[b]["fillps"] = ps
                        for k in range(KT // 2):
                            nc.tensor.matmul(ps[:], w_sb[:, k, :], xt[:, k, :],
                                             start=(k == 0), stop=False)

                    def u_fill_b(sc, which, w_sb, b_sb, dst_kind):
                        xt = st[b][f"xt{sc}"]
                        ps = st[b]["fillps"]
                        for k in range(KT // 2, KT):
                            nc.tensor.matmul(ps[:], w_sb[:, k, :], xt[:, k, :],
                                             start=False, stop=(k == KT - 1))
                        if dst_kind == "v":
                            vt = vtp.tile([128, F], F32R, tag="vt",
                                          name=f"vt{b}_{sc}")
                            st[b][f"vt{sc}"] = vt
                            nc.vector.tensor_scalar_add(vt[:], ps[:], b_sb[:])
                        else:
                            dst = st[b][dst_kind]
                            nc.vector.tensor_scalar_add(
                                dst[:, sc * F:(sc + 1) * F], ps[:], b_sb[:])
                    qu = [lambda sc=sc: u_fill_a(sc, 0, wq_sb),
                          lambda sc=sc: u_fill_b(sc, 0, wq_sb, bq_sb, "qt")]
                    fu = [lambda sc=sc: u_fill_a(sc, 1, wk_sb),
                          lambda sc=sc: u_fill_b(sc, 1, wk_sb, bk_sb, "kt"),
                          lambda sc=sc: u_fill_a(sc, 2, wv_sb),
                          lambda sc=sc: u_fill_b(sc, 2, wv_sb, bv_sb, "v")]
                    if defer_qt and sc >= 1:
                        deferred.extend(qu)
                    else:
                        fu = qu + fu

                    def u_tp(sc=sc, i=0):
                        vt = st[b][f"vt{sc}"]
                        va = st[b]["va"]
                        kb = sc * (F // 128) + i
                        tp = pso.tile([128, F], F32, tag="o",
                                      name=f"tp{b}_{sc}_{i}")
                        nc.tensor.transpose(tp[:, 0:128].bitcast(F32R),
                                            vt[:, i * 128:(i + 1) * 128],
                                            ident[:])
                        dst_ap = va[:, kb, 0:130].rearrange(
                            "p (g x) -> p g x", g=2)[:, :, 0:64]
                        src_ap = tp[:, 0:128].rearrange("p (g x) -> p g x", g=2)
                        nc.vector.tensor_copy(dst_ap, src_ap)
                    for i in range(F // 128):
                        fu.append(lambda sc=sc, i=i: u_tp(sc, i))
                    fill_units.append(fu)
                # prefetch xt one chunk ahead of the fills that consume it
                units.append(dma_units[0])
                for sc in range(NSC):
                    if sc + 1 < NSC:
                        units.append(dma_units[sc + 1])
                    units.extend(fill_units[sc])
                return (units, deferred) if defer_qt else units

            # ---- phase-C unit builder (out projection tile) ----------------
            def c_unit(b, qb, jc):
                def u():
                    ctx = st[b]["ctx"]
                    ops = pso.tile([128, F], F32, tag="o",
                                   name=f"op{b}_{qb}_{jc}")
                    nc.tensor.matmul(ops[:], ctx[:, qb * 128:(qb + 1) * 128],
                                     wo_sb[:, jc * F:(jc + 1) * F],
                                     start=True, stop=True)
                    ot = op.tile([128, F], F32, tag="ot",
                                 name=f"ot{b}_{qb}_{jc}")
                    nc.vector.tensor_copy(ot[:], ops[:])
                    nc.sync.dma_start(
                        out_d[b * S + qb * 128:b * S + (qb + 1) * 128,
                              jc * F:(jc + 1) * F], ot[:])
                return u

            # ---- emission: B(b) groups with A(b+1) + C interleaved ---------
            boot, deferred_qt = a_units(0, defer_qt=True)
            boot[1]()          # first xt DMA (needs nothing)
            load_rest_of_weights()
            boot[0]()          # tile allocs + ones cols (needs `ones` loaded)
            # preload the Exp activation table while the PE does batch-0 QKV
            junk = zp.tile([1, 32], F32, tag="junk")
            nc.scalar.activation(junk[:], ident[0:1, 0:32].bitcast(F32), AF.Exp)
            for u in boot[2:]:
                u()
            a_queue = list(deferred_qt)
            c_queue = []

            for b in range(B):
                if b + 1 < B:
                    a_queue.extend(a_units(b + 1))
                st[b]["ctx"] = cxp.tile([128, S], F32R, tag="cx",
                                        name=f"cx{b}")
                qt, kt, va = st[b]["qt"], st[b]["kt"], st[b]["va"]
                ctx = st[b]["ctx"]
                gi = 0
                n_extra = len(a_queue) + 32   # C units arrive during the batch
                n_done = 0
                for qc in range(NSC):
                    qlo = qc * F
                    ct0 = psc.tile([128, F], F32, tag="ct", name=f"c0_{b}{qc}")
                    ct1 = psc.tile([128, F], F32, tag="ct", name=f"c1_{b}{qc}")
                    ct = (ct0, ct1)
                    cu = [None, None]
                    for h in range(2):
                        hp = h * 64
                        for g in range(NKB // 2):
                            sc2 = psb.tile([128, 2, F], F32, tag="sc",
                                           name=f"s{b}{qc}{h}{g}")
                            for j in range(2):
                                kb = g * 2 + j
                                nc.tensor.matmul(
                                    sc2[:, j, :],
                                    kt[hp:hp + 64, kb * 128:(kb + 1) * 128],
                                    qt[hp:hp + 64, qlo:qlo + F],
                                    start=True, stop=True)
                            at2 = apool.tile([128, 2, F], F32R, tag="at",
                                             name=f"a{b}{qc}{h}{g}")
                            nc.scalar.activation(at2[:], sc2[:], AF.Exp)
                            for j in range(2):
                                kb = g * 2 + j
                                nc.tensor.matmul(
                                    ct[h][0:65, :],
                                    va[:, kb, h * 65:h * 65 + 65],
                                    at2[:, j, :],
                                    start=(kb == 0), stop=(kb == NKB - 1))
                            if g == NKB // 2 - 1:
                                # release ct[h] early: copy ctx+Z to SBUF
                                cuh = zp.tile([65, F], F32, tag=f"cu{h}",
                                              name=f"cu{h}_{b}{qc}")
                                nc.vector.tensor_copy(cuh[:], ct[h][0:65, :])
                                cu[h] = cuh
                            # interleave pipelined units, proportionally
                            gi += 1
                            quota = (gi * n_extra) // 64
                            while n_done < quota and (a_queue or c_queue):
                                q = a_queue if (len(a_queue) * 2 >=
                                                len(c_queue)) else c_queue
                                if not q:
                                    q = c_queue if q is a_queue else a_queue
                                q.pop(0)()
                                n_done += 1
                    # normalization for this q chunk, off the PSUM path
                    zr0 = zp.tile([1, F], F32R, tag="zr0", name=f"zr0_{b}{qc}")
                    zr1 = zp.tile([1, F], F32R, tag="zr1", name=f"zr1_{b}{qc}")
                    with nc.allow_low_precision(reason="1/Z in fp32r is fine"):
                        nc.vector.reciprocal(zr0[:], cu[0][64:65, :])
                        nc.vector.reciprocal(zr1[:], cu[1][64:65, :])
                    zs0 = zp.tile([64, F], F32, tag="zs0", name=f"zs0{b}{qc}")
                    zs1 = zp.tile([64, F], F32, tag="zs1", name=f"zs1{b}{qc}")
                    nc.gpsimd.partition_broadcast(zs0[:], zr0[:].bitcast(F32))
                    nc.gpsimd.partition_broadcast(zs1[:], zr1[:].bitcast(F32))
                    nc.vector.tensor_mul(ctx[0:64, qlo:qlo + F],
                                         cu[0][0:64, :], zs0[:])
                    nc.vector.tensor_mul(ctx[64:128, qlo:qlo + F],
                                         cu[1][0:64, :], zs1[:])
                    for qb in range(qc * (NQB // NSC), (qc + 1) * (NQB // NSC)):
                        for jc in range(D // F):
                            c_queue.append(c_unit(b, qb, jc))
                # drain leftover A units before next batch needs them
                while a_queue:
                    a_queue.pop(0)()
            while c_queue:
                c_queue.pop(0)()

    nc.compile()
    return nc


def _host_inputs(x, Wq, bq, Wk, bk, Wv, bv, Wo, bo):
    x2 = np.ascontiguousarray(np.asarray(x, np.float32).reshape(SEQ, D))
    xT = np.ascontiguousarray(x2.T)
    ident = np.eye(128, dtype=np.float32)
    ind0 = np.zeros((1, 128), np.float32); ind0[0, 0:64] = 1.0
    ind1 = np.zeros((1, 128), np.float32); ind1[0, 64:128] = 1.0
    ones = np.ones((128, NKB), np.float32)
    in_maps = []
    for c in range(NCORES):
        sl = slice(c * DPC, (c + 1) * DPC)
        in_maps.append({
            "xT": xT,
            "wqT": np.ascontiguousarray((np.asarray(Wq, np.float32)[sl] / 8.0).T),
            "wkT": np.ascontiguousarray(np.asarray(Wk, np.float32)[sl].T),
            "wvT": np.ascontiguousarray(np.asarray(Wv, np.float32)[sl].T),
            "woT": np.ascontiguousarray(np.asarray(Wo, np.float32)[:, sl].T),
            "bq": (np.asarray(bq, np.float32)[sl] / 8.0).reshape(DPC, 1),
            "bk": np.asarray(bk, np.float32)[sl].reshape(DPC, 1),
            "bv": np.asarray(bv, np.float32)[sl].reshape(DPC, 1),
            "ident": ident,
            "indic0": ind0,
            "indic1": ind1,
            "ones": ones,
        })
    return in_maps


def _run(inputs, trace=False, trace_kwargs=None):
    if "nc" not in _CACHE:
        _CACHE["nc"] = _build()
    nc = _CACHE["nc"]
    in_maps = _host_inputs(**inputs)
    res = run_bass_kernel_spmd(nc, in_maps, list(range(NCORES)), trace=trace,
                               **(trace_kwargs or {}))
    acc = res.results[0]["out"].astype(np.float32).copy()
    for c in range(1, NCORES):
        acc += res.results[c]["out"]
    acc += np.asarray(inputs["bo"], np.float32)[None, :]
    return acc.reshape(B, S, D), res


def kernel(**inputs):
    out, _ = _run(inputs)
    return out


# revision 14
# speedup vs baseline: 1.0113x; 1.0113x over previous
"""Multi-head attention (B=4, S=2048, D=1024, H=16, hd=64) on 8 NeuronCores.

Tensor-parallel over heads: core c computes heads 2c, 2c+1. See kernel.py
docstring for the math. This version software-pipelines emission: batch
b+1's QKV projection and batch b's output projection are interleaved into
batch b's attention groups so the in-order PE queue has no idle phases.
"""

import sys

sys.path.insert(0, "/opt/trn_rl_repo")

import numpy as np
import concourse.bass as bass
import concourse.bacc as bacc
import concourse.mybir as mybir
import concourse.tile as tile
from concourse.bass_utils import run_bass_kernel_spmd

F32 = mybir.dt.float32
F32R = mybir.dt.float32r
AF = mybir.ActivationFunctionType

B, S, D = 4, 2048, 1024
SEQ = B * S
NCORES = 8
DPC = 128            # dims per core = 2 heads * 64
KT = D // 128        # 8 k-tiles for the QKV contraction
F = 512              # free-dim chunk
NSC = S // F         # seq chunks per batch = 4
NKB = S // 128       # key blocks per batch = 16
NQB = S // 128       # q blocks per batch = 16

_CACHE = {}


def _build():
    nc = bacc.Bacc("TRN2", target_bir_lowering=False, debug=False,
                   enable_asserts=False)

    xT_d = nc.dram_tensor("xT", [D, SEQ], F32R, kind="ExternalInput")
    wq_d = nc.dram_tensor("wqT", [D, DPC], F32R, kind="ExternalInput")
    wk_d = nc.dram_tensor("wkT", [D, DPC], F32R, kind="ExternalInput")
    wv_d = nc.dram_tensor("wvT", [D, DPC], F32R, kind="ExternalInput")
    wo_d = nc.dram_tensor("woT", [DPC, D], F32R, kind="ExternalInput")
    bq_d = nc.dram_tensor("bq", [DPC, 1], F32, kind="ExternalInput")
    bk_d = nc.dram_tensor("bk", [DPC, 1], F32, kind="ExternalInput")
    bv_d = nc.dram_tensor("bv", [DPC, 1], F32, kind="ExternalInput")
    ident_d = nc.dram_tensor("ident", [128, 128], F32R, kind="ExternalInput")
    ind0_d = nc.dram_tensor("indic0", [1, 128], F32R, kind="ExternalInput")
    ind1_d = nc.dram_tensor("indic1", [1, 128], F32R, kind="ExternalInput")
    ones_d = nc.dram_tensor("ones", [128, NKB], F32R, kind="ExternalInput")
    out_d = nc.dram_tensor("out", [SEQ, D], F32, kind="ExternalOutput")

    with tile.TileContext(nc) as tc:
        with (
            tc.tile_pool(name="wp", bufs=1) as wp,
            tc.tile_pool(name="xp", bufs=3) as xp,
            tc.tile_pool(name="qk", bufs=2) as qk,
            tc.tile_pool(name="vp", bufs=2) as vp,
            tc.tile_pool(name="vt", bufs=2) as vtp,
            tc.tile_pool(name="ap", bufs=8) as apool,
            tc.tile_pool(name="cx", bufs=2) as cxp,
            tc.tile_pool(name="zp", bufs=2) as zp,
            tc.tile_pool(name="op", bufs=6) as op,
            # PSUM bank budget (8 total): sc2 2x2 + ct 2 + shared misc 2
            tc.tile_pool(name="ps_sc", bufs=2, space=bass.MemorySpace.PSUM) as psb,
            tc.tile_pool(name="ps_ct", bufs=2, space=bass.MemorySpace.PSUM) as psc,
            tc.tile_pool(name="ps_o", bufs=2, space=bass.MemorySpace.PSUM) as pso,
        ):
            # resident weights / constants
            wq_sb = wp.tile([128, KT, DPC], F32R, tag="wq")
            wk_sb = wp.tile([128, KT, DPC], F32R, tag="wk")
            wv_sb = wp.tile([128, KT, DPC], F32R, tag="wv")
            wo_sb = wp.tile([128, D], F32R, tag="wo")
            ident = wp.tile([128, 128], F32R, tag="id")
            ind0 = wp.tile([1, 128], F32R, tag="i0")
            ind1 = wp.tile([1, 128], F32R, tag="i1")
            ones = wp.tile([128, NKB], F32R, tag="on")
            bq_sb = wp.tile([DPC, 1], F32, tag="bq")
            bk_sb = wp.tile([DPC, 1], F32, tag="bk")
            bv_sb = wp.tile([DPC, 1], F32, tag="bv")
            nc.sync.dma_start(wq_sb[:],
                              wq_d[:].rearrange("(kt p) m -> p kt m", p=128))

            xT_r = xT_d[:].rearrange("(kt p) f -> p kt f", p=128)

            def load_rest_of_weights():
                nc.sync.dma_start(wk_sb[:],
                                  wk_d[:].rearrange("(kt p) m -> p kt m", p=128))
                nc.sync.dma_start(wv_sb[:],
                                  wv_d[:].rearrange("(kt p) m -> p kt m", p=128))
                nc.sync.dma_start(bq_sb[:], bq_d[:])
                nc.sync.dma_start(bk_sb[:], bk_d[:])
                nc.sync.dma_start(bv_sb[:], bv_d[:])
                nc.sync.dma_start(ident[:], ident_d[:])
                nc.sync.dma_start(ind0[:], ind0_d[:])
                nc.sync.dma_start(ind1[:], ind1_d[:])
                nc.sync.dma_start(ones[:], ones_d[:])
                nc.sync.dma_start(wo_sb[:], wo_d[:])

            st = [dict() for _ in range(B)]   # per-batch tiles: qt/kt/vaug/ctx

            # ---- phase-A unit builders (QKV projection for batch b) --------
            def a_units(b, defer_qt=False):
                units = []
                deferred = []
                dma_units = []
                fill_units = []

                def u_start():
                    st[b]["qt"] = qk.tile([128, S], F32R, tag="qt",
                                          name=f"qt{b}")
                    st[b]["kt"] = qk.tile([128, S], F32R, tag="kt",
                                          name=f"kt{b}")
                    # per key block: [h0 d(64) | ones | h1 d(64) | ones]
                    va = vp.tile([128, NKB, 130], F32R, tag="va",
                                 name=f"va{b}")
                    st[b]["va"] = va
                    nc.vector.tensor_copy(va[:, :, 64:65], ones[:].unsqueeze(2))
                    nc.vector.tensor_copy(va[:, :, 129:130], ones[:].unsqueeze(2))
                units.append(u_start)

                for sc in range(NSC):
                    def u_dma(sc=sc):
                        xt = xp.tile([128, KT, F], F32R, tag="xt",
                                     name=f"xt{b}_{sc}")
                        st[b][f"xt{sc}"] = xt
                        lo = b * S + sc * F
                        for k in range(KT):
                            nc.sync.dma_start(xt[:, k, :],
                                              xT_r[:, k, lo:lo + F])
                    dma_units.append(u_dma)

                    def u_fill_a(sc, which, w_sb):
                        xt = st[b][f"xt{sc}"]
                        ps = pso.tile([128, F], F32, tag="o",
                                      name=f"ps{b}_{sc}_{which}")
                        st[b]["fillps"] = ps
                        for k in range(KT // 2):
                            nc.tensor.matmul(ps[:], w_sb[:, k, :], xt[:, k, :],
                                             start=(k == 0), stop=False)

                    def u_fill_b(sc, which, w_sb, b_sb, dst_kind):
                        xt = st[b][f"xt{sc}"]
                        ps = st[b]["fillps"]
                        for k in range(KT // 2, KT):
                            nc.tensor.matmul(ps[:], w_sb[:, k, :], xt[:, k, :],
                                             start=False, stop=(k == KT - 1))
                        if dst_kind == "v":
                            vt = vtp.tile([128, F], F32R, tag="vt",
                                          name=f"vt{b}_{sc}")
                            st[b][f"vt{sc}"] = vt
                            nc.vector.tensor_scalar_add(vt[:], ps[:], b_sb[:])
                        else:
                            dst = st[b][dst_kind]
                            nc.vector.tensor_scalar_add(
                                dst[:, sc * F:(sc + 1) * F], ps[:], b_sb[:])
                    qu = [lambda sc=sc: u_fill_a(sc, 0, wq_sb),
                          lambda sc=sc: u_fill_b(sc, 0, wq_sb, bq_sb, "qt")]
                    fu = [lambda sc=sc: u_fill_a(sc, 1, wk_sb),
                          lambda sc=sc: u_fill_b(sc, 1, wk_sb, bk_sb, "kt"),
                          lambda sc=sc: u_fill_a(sc, 2, wv_sb),
                          lambda sc=sc: u_fill_b(sc, 2, wv_sb, bv_sb, "v")]
                    if defer_qt and sc >= 1:
                        deferred.extend(qu)
                    else:
                        fu = qu + fu

                    def u_tp(sc=sc, i=0):
                        vt = st[b][f"vt{sc}"]
                        va = st[b]["va"]
                        kb = sc * (F // 128) + i
                        tp = pso.tile([128, F], F32, tag="o",
                                      name=f"tp{b}_{sc}_{i}")
                        nc.tensor.transpose(tp[:, 0:128].bitcast(F32R),
                                            vt[:, i * 128:(i + 1) * 128],
                                            ident[:])
                        dst_ap = va[:, kb, 0:130].rearrange(
                            "p (g x) -> p g x", g=2)[:, :, 0:64]
                        src_ap = tp[:, 0:128].rearrange("p (g x) -> p g x", g=2)
                        nc.vector.tensor_copy(dst_ap, src_ap)
                    for i in range(F // 128):
                        fu.append(lambda sc=sc, i=i: u_tp(sc, i))
                    fill_units.append(fu)
                # prefetch xt one chunk ahead of the fills that consume it
                units.append(dma_units[0])
                for sc in range(NSC):
                    if sc + 1 < NSC:
                        units.append(dma_units[sc + 1])
                    units.extend(fill_units[sc])
                return (units, deferred) if defer_qt else units

            # ---- phase-C unit builder (out projection tile) ----------------
            def c_unit(b, qb, jc):
                def u():
                    ctx = st[b]["ctx"]
                    ops = pso.tile([128, F], F32, tag="o",
                                   name=f"op{b}_{qb}_{jc}")
                    nc.tensor.matmul(ops[:], ctx[:, qb * 128:(qb + 1) * 128],
                                     wo_sb[:, jc * F:(jc + 1) * F],
                                     start=True, stop=True)
                    ot = op.tile([128, F], F32, tag="ot",
                                 name=f"ot{b}_{qb}_{jc}")
                    nc.vector.tensor_copy(ot[:], ops[:])
                    nc.sync.dma_start(
                        out_d[b * S + qb * 128:b * S + (qb + 1) * 128,
                              jc * F:(jc + 1) * F], ot[:])
                return u

            # ---- emission: B(b) groups with A(b+1) + C interleaved ---------
            boot, deferred_qt = a_units(0, defer_qt=True)
            boot[1]()          # first xt DMA (needs nothing)
            load_rest_of_weights()
            boot[0]()          # tile allocs + ones cols (needs `ones` loaded)
            # preload the Exp activation table while the PE does batch-0 QKV
            junk = zp.tile([1, 32], F32, tag="junk")
            nc.scalar.activation(junk[:], ident[0:1, 0:32].bitcast(F32), AF.Exp)
            for u in boot[2:]:
                u()
            a_queue = list(deferred_qt)
            c_queue = []

            for b in range(B):
                if b + 1 < B:
                    a_queue.extend(a_units(b + 1))
                st[b]["ctx"] = cxp.tile([128, S], F32R, tag="cx",
                                        name=f"cx{b}")
                qt, kt, va = st[b]["qt"], st[b]["kt"], st[b]["va"]
                ctx = st[b]["ctx"]
                gi = 0
                n_extra = len(a_queue) + 32   # C units arrive during the batch
                n_done = 0
                for qc in range(NSC):
                    qlo = qc * F
                    ct0 = psc.tile([128, F], F32, tag="ct", name=f"c0_{b}{qc}")
                    ct1 = psc.tile([128, F], F32, tag="ct", name=f"c1_{b}{qc}")
                    ct = (ct0, ct1)
                    cu = [None, None]
                    for h in range(2):
                        hp = h * 64
                        for g in range(NKB // 2):
                            sc2 = psb.tile([128, 2, F], F32, tag="sc",
                                           name=f"s{b}{qc}{h}{g}")
                            for j in range(2):
                                kb = g * 2 + j
                                nc.tensor.matmul(
                                    sc2[:, j, :],
                                    kt[hp:hp + 64, kb * 128:(kb + 1) * 128],
                                    qt[hp:hp + 64, qlo:qlo + F],
                                    start=True, stop=True)
                            at2 = apool.tile([128, 2, F], F32R, tag="at",
                                             name=f"a{b}{qc}{h}{g}")
                            nc.scalar.activation(at2[:], sc2[:], AF.Exp)
                            for j in range(2):
                                kb = g * 2 + j
                                nc.tensor.matmul(
                                    ct[h][0:65, :],
                                    va[:, kb, h * 65:h * 65 + 65],
                                    at2[:, j, :],
                                    start=(kb == 0), stop=(kb == NKB - 1))
                            if g == NKB // 2 - 1:
                                # release ct[h] early: copy ctx+Z to SBUF
                                cuh = zp.tile([65, F], F32, tag=f"cu{h}",
                                              name=f"cu{h}_{b}{qc}")
                                nc.vector.tensor_copy(cuh[:], ct[h][0:65, :])
                                cu[h] = cuh
                            # interleave pipelined units, proportionally
                            gi += 1
                            quota = (gi * n_extra) // 64
                            while n_done < quota and (a_queue or c_queue):
                                q = a_queue if (len(a_queue) * 2 >=
                                                len(c_queue)) else c_queue
                                if not q:
                                    q = c_queue if q is a_queue else a_queue
                                q.pop(0)()
                                n_done += 1
                    # normalization for this q chunk, off the PSUM path
                    zr0 = zp.tile([1, F], F32R, tag="zr0", name=f"zr0_{b}{qc}")
                    zr1 = zp.tile([1, F], F32R, tag="zr1", name=f"zr1_{b}{qc}")
                    with nc.allow_low_precision(reason="1/Z in fp32r is fine"):
                        nc.vector.reciprocal(zr0[:], cu[0][64:65, :])
                        nc.vector.reciprocal(zr1[:], cu[1][64:65, :])
                    zs0 = zp.tile([64, F], F32, tag="zs0", name=f"zs0{b}{qc}")
                    zs1 = zp.tile([64, F], F32, tag="zs1", name=f"zs1{b}{qc}")
                    nc.gpsimd.partition_broadcast(zs0[:], zr0[:].bitcast(F32))
                    nc.gpsimd.partition_broadcast(zs1[:], zr1[:].bitcast(F32))
                    nc.vector.tensor_mul(ctx[0:64, qlo:qlo + F],
                                         cu[0][0:64, :], zs0[:])
                    nc.vector.tensor_mul(ctx[64:128, qlo:qlo + F],
                                         cu[1][0:64, :], zs1[:])
                    for qb in range(qc * (NQB // NSC), (qc + 1) * (NQB // NSC)):
                        for jc in range(D // F):
                            c_queue.append(c_unit(b, qb, jc))
                # drain leftover A units before next batch needs them
                while a_queue:
                    a_queue.pop(0)()
            while c_queue:
                c_queue.pop(0)()

    nc.compile()
    return nc


def _host_inputs(x, Wq, bq, Wk, bk, Wv, bv, Wo, bo):
    x2 = np.ascontiguousarray(np.asarray(x, np.float32).reshape(SEQ, D))
    xT = np.ascontiguousarray(x2.T)
    ident = np.eye(128, dtype=np.float32)
    ind0 = np.zeros((1, 128), np.float32); ind0[0, 0:64] = 1.0
    ind1 = np.zeros((1, 128), np.float32); ind1[0, 64:128] = 1.0
    ones = np.ones((128, NKB), np.float32)
    in_maps = []
    for c in range(NCORES):
        sl = slice(c * DPC, (c + 1) * DPC)
        in_maps.append({
            "xT": xT,
            "wqT": np.ascontiguousarray((np.asarray(Wq, np.float32)[sl] / 8.0).T),
            "wkT": np.ascontiguousarray(np.asarray(Wk, np.float32)[sl].T),
            "wvT": np.ascontiguousarray(np.asarray(Wv, np.float32)[sl].T),
            "woT": np.ascontiguousarray(np.asarray(Wo, np.float32)[:, sl].T),
            "bq": (np.asarray(bq, np.float32)[sl] / 8.0).reshape(DPC, 1),
            "bk": np.asarray(bk, np.float32)[sl].reshape(DPC, 1),
            "bv": np.asarray(bv, np.float32)[sl].reshape(DPC, 1),
            "ident": ident,
            "indic0": ind0,
            "indic1": ind1,
            "ones": ones,
        })
    return in_maps


def _run(inputs, trace=False, trace_kwargs=None):
    if "nc" not in _CACHE:
        _CACHE["nc"] = _build()
    nc = _CACHE["nc"]
    in_maps = _host_inputs(**inputs)
    res = run_bass_kernel_spmd(nc, in_maps, list(range(NCORES)), trace=trace,
                               **(trace_kwargs or {}))
    acc = res.results[0]["out"].astype(np.float32).copy()
    for c in range(1, NCORES):
        acc += res.results[c]["out"]
    acc += np.asarray(inputs["bo"], np.float32)[None, :]
    return acc.reshape(B, S, D), res


def kernel(**inputs):
    out, _ = _run(inputs)
    return out
